# revision 3
# baseline (speedup 1.0000x reference)
"""Bass/Tile TRN2 kernel for nn_Actor_DeepSet (8-core data parallel).

Reference computation (per row r = b*8 + i, obs=64, hidden=128):
  h1   = relu(x_r @ w1.T + b1)
  hsum = (1/8) * sum_{k=1..7} relu(rot_{i+1}(x_{b,k}) @ w1o.T + b1o)
  h2   = relu([h1, hsum] @ w2.T + b2)
  out  = h2 @ wv.T + bv
where rot_s rotates the 64 features (jnp.roll(x, -s, axis=1)), equivalently a
column rotation of w1o.  The 1/8 folds into w1o/b1o (relu pos. homogeneous).

Device layout: transposed (channels on partitions, rows on free axis).
Per core: x.T [65, 16384] bf16 (row 64 = ones so biases ride the matmul),
32 tiles of 512 rows (64 batches).  Output y.T [16, 16384] f32.
"""

import os
import numpy as np

import concourse.bacc as bacc
import concourse.mybir as mybir
import concourse.tile as tile
from concourse.bass_utils import run_bass_kernel_spmd

N_CORES = 8
N_AGENTS = 8
OBS = 64
HIDDEN = 128
NUM_OUT = 16
ROWS_PC = 16384            # rows per core
TILE_N = 512               # rows per tile
N_TILES = ROWS_PC // TILE_N
NB = TILE_N // N_AGENTS    # batches per tile (64)

BF16 = mybir.dt.bfloat16
F32 = mybir.dt.float32
NP_BF16 = mybir.dt.np(BF16)

AF = mybir.ActivationFunctionType

_compiled_nc = None
last_exec_time_ns = None


def _build_nc():
    nc = bacc.Bacc("TRN2", target_bir_lowering=False, debug=False,
                   num_devices=N_CORES)

    x_ext = nc.dram_tensor("x", [OBS + 1, ROWS_PC], BF16, kind="ExternalInput")
    wl1_ext = nc.dram_tensor("wl1", [OBS + 1, HIDDEN], BF16, kind="ExternalInput")
    wcat_ext = nc.dram_tensor("wcat", [N_AGENTS, OBS + 1, HIDDEN], BF16,
                              kind="ExternalInput")
    w2a_ext = nc.dram_tensor("w2a", [HIDDEN, HIDDEN], BF16, kind="ExternalInput")
    w2b_ext = nc.dram_tensor("w2b", [HIDDEN, HIDDEN], BF16, kind="ExternalInput")
    wv_ext = nc.dram_tensor("wv", [HIDDEN, NUM_OUT], BF16, kind="ExternalInput")
    b2_ext = nc.dram_tensor("b2", [HIDDEN, 1], F32, kind="ExternalInput")
    bv_ext = nc.dram_tensor("bv", [NUM_OUT, 1], F32, kind="ExternalInput")
    y_ext = nc.dram_tensor("y", [NUM_OUT, ROWS_PC], F32, kind="ExternalOutput")

    with tile.TileContext(nc) as tc:
        with (
            tc.tile_pool(name="const", bufs=1) as cpool,
            tc.tile_pool(name="xin", bufs=3) as xpool,
            tc.tile_pool(name="act", bufs=3) as apool,
            tc.tile_pool(name="rbuf", bufs=2) as rpool,
            tc.tile_pool(name="outb", bufs=3) as opool,
            tc.tile_pool(name="ps_mm", bufs=4, space="PSUM") as pmm,
            tc.tile_pool(name="ps_s", bufs=4, space="PSUM") as pss,
        ):
            # --- persistent weights ---
            wl1 = cpool.tile([OBS + 1, HIDDEN], BF16)
            nc.sync.dma_start(wl1[:], wl1_ext[:])
            wcat = cpool.tile([OBS + 1, N_AGENTS * HIDDEN], BF16)
            for s in range(N_AGENTS):
                nc.sync.dma_start(wcat[:, s * HIDDEN:(s + 1) * HIDDEN],
                                  wcat_ext[s])
            w2a = cpool.tile([HIDDEN, HIDDEN], BF16)
            nc.sync.dma_start(w2a[:], w2a_ext[:])
            w2b = cpool.tile([HIDDEN, HIDDEN], BF16)
            nc.sync.dma_start(w2b[:], w2b_ext[:])
            wv = cpool.tile([HIDDEN, NUM_OUT], BF16)
            nc.sync.dma_start(wv[:], wv_ext[:])
            b2t = cpool.tile([HIDDEN, 1], F32)
            nc.sync.dma_start(b2t[:], b2_ext[:])
            bvt = cpool.tile([NUM_OUT, 1], F32)
            nc.sync.dma_start(bvt[:], bv_ext[:])

            for t in range(N_TILES):
                # x.T tile [65, 512]; cols ordered r = b*8 + a
                xt = xpool.tile([OBS + 1, TILE_N], BF16)
                nc.sync.dma_start(xt[:], x_ext[:, t * TILE_N:(t + 1) * TILE_N])
                # view [65, a=8, b=64]; agents 1..7 -> moving N = 7*64,
                # psum col = k*64 + b (k = a-1 outer)
                x_ab = xt[:].rearrange("p (b a) -> p a b", a=N_AGENTS)
                x_other = x_ab[:, 1:N_AGENTS, :]

                # --- layer 1, self part ---
                ps1 = pmm.tile([HIDDEN, TILE_N], F32, tag="mm")
                nc.tensor.matmul(ps1[:], wl1[:], xt[:])
                htop = apool.tile([HIDDEN, TILE_N], BF16)
                nc.scalar.activation(htop[:], ps1[:], AF.Relu)

                # --- layer 1, others: 8 shifted weights over agents 1..7 ---
                # r layout: [128, k=7, s=8, b=64]
                r = rpool.tile([HIDDEN, 7 * N_AGENTS * NB], BF16)
                r_v = r[:].rearrange("p (k s b) -> p k s b", k=7, s=N_AGENTS)
                for s in range(N_AGENTS):
                    ps = pss.tile([HIDDEN, 7 * NB], F32)
                    nc.tensor.matmul(ps[:], wcat[:, s * HIDDEN:(s + 1) * HIDDEN],
                                     x_other)
                    nc.scalar.activation(
                        r_v[:, :, s, :],
                        ps[:].rearrange("p (k b) -> p k b", k=7),
                        AF.Relu,
                    )

                # --- segment sum over k (7 slabs of [128, 512], bf16 2x) ---
                hbot = apool.tile([HIDDEN, N_AGENTS * NB], BF16)  # (s, b) order
                r_k = r[:].rearrange("p (k c) -> p k c", k=7)
                with nc.allow_low_precision("bf16 partial sums"):
                    nc.vector.tensor_add(hbot[:], r_k[:, 0, :], r_k[:, 1, :])
                    for k in range(2, 7):
                        nc.vector.tensor_add(hbot[:], hbot[:], r_k[:, k, :])

                # --- layer 2 ---
                ps2 = pmm.tile([HIDDEN, TILE_N], F32, tag="mm")
                nc.tensor.matmul(ps2[:], w2a[:], htop[:], start=True, stop=False)
                # moving view reorders (s,b) -> col j = b*8 + s to match rows
                hbot_bs = hbot[:].rearrange("p (s b) -> p b s", s=N_AGENTS)
                nc.tensor.matmul(ps2[:], w2b[:], hbot_bs, start=False, stop=True)
                h2 = apool.tile([HIDDEN, TILE_N], BF16)
                nc.scalar.activation(h2[:], ps2[:], AF.Relu, bias=b2t[:])

                # --- layer 3 ---
                ps3 = pmm.tile([NUM_OUT, TILE_N], F32, tag="mm")
                nc.tensor.matmul(ps3[:], wv[:], h2[:])
                o = opool.tile([NUM_OUT, TILE_N], F32)
                nc.scalar.activation(o[:], ps3[:], AF.Identity, bias=bvt[:])
                nc.sync.dma_start(y_ext[:, t * TILE_N:(t + 1) * TILE_N], o[:])

    nc.compile()
    return nc


def kernel(inputs, w1, b1, w1o, b1o, w2, b2, wv, bv):
    global _compiled_nc, last_exec_time_ns
    if _compiled_nc is None:
        _compiled_nc = _build_nc()
    nc = _compiled_nc

    inputs = np.asarray(inputs, dtype=np.float32)
    w1 = np.asarray(w1, dtype=np.float32)
    b1 = np.asarray(b1, dtype=np.float32)
    w1o = np.asarray(w1o, dtype=np.float32)
    b1o = np.asarray(b1o, dtype=np.float32)
    w2 = np.asarray(w2, dtype=np.float32)
    b2 = np.asarray(b2, dtype=np.float32)
    wv = np.asarray(wv, dtype=np.float32)
    bv = np.asarray(bv, dtype=np.float32)

    # host-side weight prep (tiny)
    wl1 = np.concatenate([w1.T, b1[None, :]], axis=0).astype(NP_BF16)  # [65,128]
    wcat = np.empty((N_AGENTS, OBS + 1, HIDDEN), dtype=NP_BF16)
    for si in range(N_AGENTS):
        s = si + 1  # shift amount for agent i = si
        lhsT = np.roll(w1o, s, axis=1).T / N_AGENTS       # [64, 128]
        wcat[si, :OBS] = lhsT.astype(NP_BF16)
        wcat[si, OBS] = (b1o / N_AGENTS).astype(NP_BF16)
    w2a = np.ascontiguousarray(w2[:, :HIDDEN].T).astype(NP_BF16)
    w2b = np.ascontiguousarray(w2[:, HIDDEN:].T).astype(NP_BF16)
    wvt = np.ascontiguousarray(wv.T).astype(NP_BF16)
    b2c = np.ascontiguousarray(b2[:, None]).astype(np.float32)
    bvc = np.ascontiguousarray(bv[:, None]).astype(np.float32)

    # shard rows across cores; feed x.T with a ones row appended
    xs = inputs.reshape(N_CORES, ROWS_PC, OBS)
    in_maps = []
    for c in range(N_CORES):
        xT = np.empty((OBS + 1, ROWS_PC), dtype=NP_BF16)
        xT[:OBS] = xs[c].T.astype(NP_BF16)
        xT[OBS] = np.ones((ROWS_PC,), dtype=NP_BF16)
        in_maps.append({
            "x": xT, "wl1": wl1, "wcat": wcat, "w2a": w2a, "w2b": w2b,
            "wv": wvt, "b2": b2c, "bv": bvc,
        })

    trace = bool(int(os.environ.get("BASS_KERNEL_TRACE", "0")))
    res = run_bass_kernel_spmd(nc, in_maps, list(range(N_CORES)), trace=trace)
    last_exec_time_ns = res.exec_time_ns

    y = np.stack([res.results[c]["y"] for c in range(N_CORES)])  # [8,16,16384]
    out = y.transpose(0, 2, 1).reshape(N_CORES * ROWS_PC, NUM_OUT)
    return np.ascontiguousarray(out, dtype=np.float32)


# revision 5
# speedup vs baseline: 1.4310x; 1.4310x over previous
"""Bass/Tile TRN2 kernel for nn_Actor_DeepSet (8-core data parallel).

Reference computation (per row r = b*8 + i, obs=64, hidden=128):
  h1   = relu(x_r @ w1.T + b1)
  hsum = (1/8) * sum_{k=1..7} relu(rot_{i+1}(x_{b,k}) @ w1o.T + b1o)
  h2   = relu([h1, hsum] @ w2.T + b2)
  out  = h2 @ wv.T + bv
where rot_s rotates the 64 features (jnp.roll(x, -s, axis=1)), equivalently a
column rotation of w1o.  The 1/8 folds into w1o/b1o (relu pos. homogeneous).

Device layout: transposed (channels on partitions, rows on free axis), bf16
compute, f32 PSUM.  Each 512-row tile is reordered agent-major on the host:
tile column j = a*64 + b (a = agent 0..7, b = batch 0..63), so the
"other agents" moving operand is simply columns 64..511 (contiguous) and all
matmul moving operands are dense.  The k-th "other agent" slab of the layer-1
products lands k-major in PSUM; relu'd slabs are stored [128, k=7, s=8, b=64]
in SBUF; the sum over k runs partly as DVE adds and partly folded into the
layer-2 PSUM accumulation (extra w2b matmuls).  Output y.T [16, 16384] in
tile-(a,b) order; host unscrambles.
"""

import os
import numpy as np

import concourse.bacc as bacc
import concourse.mybir as mybir
import concourse.tile as tile
from concourse.bass_utils import run_bass_kernel_spmd

N_CORES = 8
N_AGENTS = 8
OBS = 64
HIDDEN = 128
NUM_OUT = 16
ROWS_PC = 16384            # rows per core
TILE_N = 512               # rows per tile
N_TILES = ROWS_PC // TILE_N
NB = TILE_N // N_AGENTS    # batches per tile (64)

# tuning knobs
N_FOLD = int(os.environ.get("KN_FOLD", "3"))        # k-slabs folded into L2 PSUM
N_DVE_SHIFTS = int(os.environ.get("KN_DVE_SHIFTS", "1"))  # shift drains on DVE

BF16 = mybir.dt.bfloat16
F32 = mybir.dt.float32
NP_BF16 = mybir.dt.np(BF16)

AF = mybir.ActivationFunctionType

_compiled_nc = None
last_exec_time_ns = None


def _build_nc():
    nc = bacc.Bacc("TRN2", target_bir_lowering=False, debug=False,
                   num_devices=N_CORES)

    x_ext = nc.dram_tensor("x", [OBS + 1, ROWS_PC], BF16, kind="ExternalInput")
    wl1_ext = nc.dram_tensor("wl1", [OBS + 1, HIDDEN], BF16, kind="ExternalInput")
    wcat_ext = nc.dram_tensor("wcat", [N_AGENTS, OBS + 1, HIDDEN], BF16,
                              kind="ExternalInput")
    w2a_ext = nc.dram_tensor("w2a", [HIDDEN, HIDDEN], BF16, kind="ExternalInput")
    w2b_ext = nc.dram_tensor("w2b", [HIDDEN, HIDDEN], BF16, kind="ExternalInput")
    wv_ext = nc.dram_tensor("wv", [HIDDEN, NUM_OUT], BF16, kind="ExternalInput")
    b2_ext = nc.dram_tensor("b2", [HIDDEN, 1], F32, kind="ExternalInput")
    bv_ext = nc.dram_tensor("bv", [NUM_OUT, 1], F32, kind="ExternalInput")
    y_ext = nc.dram_tensor("y", [NUM_OUT, ROWS_PC], F32, kind="ExternalOutput")

    with tile.TileContext(nc) as tc:
        with (
            tc.tile_pool(name="const", bufs=1) as cpool,
            tc.tile_pool(name="xin", bufs=3) as xpool,
            tc.tile_pool(name="act", bufs=3) as apool,
            tc.tile_pool(name="rbuf", bufs=2) as rpool,
            tc.tile_pool(name="outb", bufs=3) as opool,
            tc.tile_pool(name="ps_mm", bufs=4, space="PSUM") as pmm,
            tc.tile_pool(name="ps_s", bufs=4, space="PSUM") as pss,
        ):
            # --- persistent weights ---
            wl1 = cpool.tile([OBS + 1, HIDDEN], BF16)
            nc.sync.dma_start(wl1[:], wl1_ext[:])
            wcat = cpool.tile([OBS + 1, N_AGENTS * HIDDEN], BF16)
            for s in range(N_AGENTS):
                nc.sync.dma_start(wcat[:, s * HIDDEN:(s + 1) * HIDDEN],
                                  wcat_ext[s])
            w2a = cpool.tile([HIDDEN, HIDDEN], BF16)
            nc.sync.dma_start(w2a[:], w2a_ext[:])
            w2b = cpool.tile([HIDDEN, HIDDEN], BF16)
            nc.sync.dma_start(w2b[:], w2b_ext[:])
            wv = cpool.tile([HIDDEN, NUM_OUT], BF16)
            nc.sync.dma_start(wv[:], wv_ext[:])
            b2t = cpool.tile([HIDDEN, 1], F32)
            nc.sync.dma_start(b2t[:], b2_ext[:])
            bvt = cpool.tile([NUM_OUT, 1], F32)
            nc.sync.dma_start(bvt[:], bv_ext[:])

            n_tt = 6 - N_FOLD  # DVE adds over k-slabs 0..6-N_FOLD

            for t in range(N_TILES):
                # x.T tile [65, 512]; cols ordered j = a*64 + b
                xt = xpool.tile([OBS + 1, TILE_N], BF16)
                nc.sync.dma_start(xt[:], x_ext[:, t * TILE_N:(t + 1) * TILE_N])

                # --- layer 1, self part ---
                ps1 = pmm.tile([HIDDEN, TILE_N], F32, tag="mm")
                nc.tensor.matmul(ps1[:], wl1[:], xt[:])
                htop = apool.tile([HIDDEN, TILE_N], BF16)
                nc.scalar.activation(htop[:], ps1[:], AF.Relu)

                # --- layer 1, others ---
                # moving operand: agents 1..7 = cols 64..511 (contiguous);
                # psum col = k*64 + b.  r layout: [128, k=7, s=8, b=64]
                r = rpool.tile([HIDDEN, 7 * N_AGENTS * NB], BF16)
                r_v = r[:].rearrange("p (k s b) -> p k s b", k=7, s=N_AGENTS)
                x_other = xt[:, NB:TILE_N]
                for s in range(N_AGENTS):
                    ps = pss.tile([HIDDEN, 7 * NB], F32)
                    nc.tensor.matmul(ps[:], wcat[:, s * HIDDEN:(s + 1) * HIDDEN],
                                     x_other)
                    dst = r_v[:, :, s, :]
                    src = ps[:].rearrange("p (k b) -> p k b", k=7)
                    if s < N_DVE_SHIFTS:
                        nc.vector.tensor_scalar_max(dst, src, 0.0)
                    else:
                        nc.scalar.activation(dst, src, AF.Relu)

                # --- partial segment sum over k-slabs 0..6-N_FOLD (DVE) ---
                r_k = r[:].rearrange("p (k c) -> p k c", k=7)
                hbot = apool.tile([HIDDEN, N_AGENTS * NB], BF16)  # (s, b) order
                with nc.allow_low_precision("bf16 partial sums"):
                    if n_tt == 0:
                        hbot = None
                    else:
                        nc.vector.tensor_add(hbot[:], r_k[:, 0, :], r_k[:, 1, :])
                        for k in range(2, n_tt + 1):
                            nc.vector.tensor_add(hbot[:], hbot[:], r_k[:, k, :])

                # --- layer 2 (PSUM accumulation; folded slabs ride along) ---
                ps2 = pmm.tile([HIDDEN, TILE_N], F32, tag="mm")
                nc.tensor.matmul(ps2[:], w2a[:], htop[:], start=True, stop=False)
                if hbot is not None:
                    nc.tensor.matmul(ps2[:], w2b[:], hbot[:],
                                     start=False, stop=False)
                first_fold = 7 - N_FOLD if hbot is not None else 0
                for k in range(first_fold, 7):
                    nc.tensor.matmul(ps2[:], w2b[:], r_k[:, k, :],
                                     start=False, stop=(k == 6))
                h2 = apool.tile([HIDDEN, TILE_N], BF16)
                nc.vector.tensor_scalar(h2[:], ps2[:], b2t[:], 0.0,
                                        mybir.AluOpType.add,
                                        mybir.AluOpType.max)

                # --- layer 3 ---
                ps3 = pmm.tile([NUM_OUT, TILE_N], F32, tag="mm")
                nc.tensor.matmul(ps3[:], wv[:], h2[:])
                o = opool.tile([NUM_OUT, TILE_N], F32)
                nc.scalar.activation(o[:], ps3[:], AF.Identity, bias=bvt[:])
                nc.sync.dma_start(y_ext[:, t * TILE_N:(t + 1) * TILE_N], o[:])

    nc.compile()
    return nc


def kernel(inputs, w1, b1, w1o, b1o, w2, b2, wv, bv):
    global _compiled_nc, last_exec_time_ns
    if _compiled_nc is None:
        _compiled_nc = _build_nc()
    nc = _compiled_nc

    inputs = np.asarray(inputs, dtype=np.float32)
    w1 = np.asarray(w1, dtype=np.float32)
    b1 = np.asarray(b1, dtype=np.float32)
    w1o = np.asarray(w1o, dtype=np.float32)
    b1o = np.asarray(b1o, dtype=np.float32)
    w2 = np.asarray(w2, dtype=np.float32)
    b2 = np.asarray(b2, dtype=np.float32)
    wv = np.asarray(wv, dtype=np.float32)
    bv = np.asarray(bv, dtype=np.float32)

    # host-side weight prep (tiny)
    wl1 = np.concatenate([w1.T, b1[None, :]], axis=0).astype(NP_BF16)  # [65,128]
    wcat = np.empty((N_AGENTS, OBS + 1, HIDDEN), dtype=NP_BF16)
    for si in range(N_AGENTS):
        s = si + 1  # shift amount for agent i = si
        lhsT = np.roll(w1o, s, axis=1).T / N_AGENTS       # [64, 128]
        wcat[si, :OBS] = lhsT.astype(NP_BF16)
        wcat[si, OBS] = (b1o / N_AGENTS).astype(NP_BF16)
    w2a = np.ascontiguousarray(w2[:, :HIDDEN].T).astype(NP_BF16)
    w2b = np.ascontiguousarray(w2[:, HIDDEN:].T).astype(NP_BF16)
    wvt = np.ascontiguousarray(wv.T).astype(NP_BF16)
    b2c = np.ascontiguousarray(b2[:, None]).astype(np.float32)
    bvc = np.ascontiguousarray(bv[:, None]).astype(np.float32)

    # shard rows across cores; x.T columns reordered per tile to (a, b)
    xs = inputs.reshape(N_CORES, N_TILES, NB, N_AGENTS, OBS)
    # -> [core, obs, tile, agent, batch]
    xs_t = xs.transpose(0, 4, 1, 3, 2).reshape(N_CORES, OBS, ROWS_PC)
    in_maps = []
    for c in range(N_CORES):
        xT = np.empty((OBS + 1, ROWS_PC), dtype=NP_BF16)
        xT[:OBS] = xs_t[c].astype(NP_BF16)
        xT[OBS] = np.ones((ROWS_PC,), dtype=NP_BF16)
        in_maps.append({
            "x": xT, "wl1": wl1, "wcat": wcat, "w2a": w2a, "w2b": w2b,
            "wv": wvt, "b2": b2c, "bv": bvc,
        })

    trace = bool(int(os.environ.get("BASS_KERNEL_TRACE", "0")))
    res = run_bass_kernel_spmd(nc, in_maps, list(range(N_CORES)), trace=trace)
    last_exec_time_ns = res.exec_time_ns

    y = np.stack([res.results[c]["y"] for c in range(N_CORES)])  # [8,16,16384]
    # y columns are (tile, agent, batch); rows are (tile, batch, agent)
    y = y.reshape(N_CORES, NUM_OUT, N_TILES, N_AGENTS, NB)
    out = y.transpose(0, 2, 4, 3, 1).reshape(N_CORES * ROWS_PC, NUM_OUT)
    return np.ascontiguousarray(out, dtype=np.float32)
